# revision 1
# baseline (speedup 1.0000x reference)
"""DAG-constraint layer kernel for Trainium2 (8 NeuronCores, data parallel).

The reference computes p = sigmoid(x) followed by an iterative min/max
projection over a fixed chain+skip DAG on N=32 nodes (children of i are
{i+1, i+2}).  On that DAG the projection's fixed point is reached after a
single iteration and collapses to the prefix-min along the node axis:

    out[b, j] = min_{k <= j} sigmoid(x[b, k]) = sigmoid(cummin(x, axis=1))

(verified bitwise against the reference).  So the kernel is a per-row
prefix-min over 32 columns plus a sigmoid - purely memory bound.

Per core: rows are sharded 8 ways (65536 rows x 32 f32 = 8 MiB per shard).
The shard is processed as [128 partitions x F free] tiles; each partition
holds F/32 complete rows, so each row's 32 columns are contiguous in the
free dimension.  The prefix-min of many rows is computed with one hardware
scan instruction (TensorTensorScanArith) per tile:

    state_t = max( min(x_t, state_{t-1}), C_t )

where C is a constant: +BIG at each row's LAST column (t % 32 == 31) and
-BIG elsewhere.  The +BIG poisons the state at each row end, so the next
row starts a fresh running min (initial=+BIG handles the first row).  Each
row's column 31 then holds +BIG instead of the true value; one cheap
strided min (64 elements/partition) repairs it:
    q[:, 31::32] = min(q[:, 30::32], x[:, 31::32])
Sigmoid runs on the scalar engine in place.

Raw bass (explicit semaphores) rather than Tile: the walrus build in this
container only encodes a single sync-wait per instruction, so waits are
issued as standalone wait_ge commands.  Pipeline: sync engine issues input
DMAs (plus a gated SWDGE prefetch of the tail tiles on gpsimd, a third DMA
ring), vector (DVE) runs scan+fix, scalar (ACT) runs sigmoid and issues
output DMAs.  Per-tile input semaphores give exact completion; the single
output semaphore is only waited at its total.

kernel() runs in-process when the 8 NeuronCores are visible to jax;
otherwise (e.g. the caller pinned jax to CPU) it re-executes itself in a
clean subprocess.
"""

import os
import subprocess
import sys
import tempfile
from contextlib import ExitStack

import numpy as np

import concourse.bass as bass
import concourse.mybir as mybir
from concourse.bass_utils import run_bass_kernel_spmd

N_CORES = 8
B_TOTAL = 524288
N_NODES = 32
ROWS_PER_CORE = B_TOTAL // N_CORES  # 65536
P = 128                             # SBUF partitions
# Per-tile free-dim sizes (f32 elements per partition).  Small tiles at the
# head shorten the pipeline fill (first scan can start ~3us earlier);
# moderate tiles at the tail shorten the drain (last scan->sigmoid->store
# chain) while staying >= 1024 so their column-31 fix can run immediately
# after the scan (see the hazard note in the vector block).
FSIZES = [512, 512, 1024] + [2048] * 6 + [1024, 1024]
FMAX = max(FSIZES)
NT = len(FSIZES)
NEG_BIG = -3.0e38
POS_BIG = 3.0e38

assert sum(FSIZES) * P == ROWS_PER_CORE * N_NODES
assert all(f % N_NODES == 0 for f in FSIZES)


def _col(ap, c):
    """AP selecting column c of every N_NODES-wide row: [P, F/N] stride N."""
    return ap[:].rearrange("p (g n) -> p g n", n=N_NODES)[:, :, c]


def _build() -> bass.Bass:
    nc = bass.Bass()
    f32 = mybir.dt.float32
    x = nc.declare_dram_parameter("x", [ROWS_PER_CORE, N_NODES], f32, isOutput=False)
    y = nc.declare_dram_parameter("y", [ROWS_PER_CORE, N_NODES], f32, isOutput=True)
    xf = x[:].flatten()
    yf = y[:].flatten()
    # DRAM chunk per tile t: contiguous [P, FSIZES[t]] starting at offset[t]
    offs = [0]
    for fsz in FSIZES:
        offs.append(offs[-1] + P * fsz)

    def _dram_tile(flat, t):
        return flat[offs[t] : offs[t + 1]].rearrange("(p f) -> p f", p=P)

    with ExitStack() as es:
        ec = es.enter_context
        # All NT tiles resident at once (17 MiB of SBUF): no slot reuse, so
        # the input DMA stream runs with no dependency on compute at all.
        xts = [ec(nc.sbuf_tensor(f"xt{i}", [P, FSIZES[i]], f32)) for i in range(NT)]
        qts = [ec(nc.sbuf_tensor(f"qt{i}", [P, FSIZES[i]], f32)) for i in range(NT)]
        cmask = ec(nc.sbuf_tensor("cmask", [P, FMAX], f32))
        warm = ec(nc.sbuf_tensor("act_warm", [P, 1], f32))
        sep = ec(nc.sbuf_tensor("sep", [P, 64], f32))
        # Per-tile input semaphores: a cumulative count over several
        # in-flight DMAs is NOT a completion indicator (the 16 per-SDMA-
        # engine increments of different DMAs interleave), but with one DMA
        # per semaphore the count is exact.  The single output semaphore is
        # only ever waited at its total (all increments fired), so a shared
        # counter is fine there.
        dma_in = [ec(nc.semaphore(f"dma_in{i}")) for i in range(NT)]
        dma_out = ec(nc.semaphore("dma_out"))
        scan_sem = ec(nc.semaphore("scan_sem"))
        gp_sem = ec(nc.semaphore("gp_sem"))
        act_sem = ec(nc.semaphore("act_sem"))

        with nc.Block() as block:

            # The scan consumes input at ~246 GB/s while the shared SP ring
            # delivers ~236 GB/s mid-kernel - the tail tiles would arrive
            # just too late.  Ship the last two tiles through the separate
            # SWDGE (gpsimd) ring up front so they are resident early.
            SWDGE_TILES = {NT - 2, NT - 1}

            @block.sync
            def _(sync):
                for t in range(NT):
                    if t in SWDGE_TILES:
                        continue
                    sync.dma_start(
                        out=xts[t][:], in_=_dram_tile(xf, t)
                    ).then_inc(dma_in[t], 16)

            @block.gpsimd
            def _(gp):
                # Wait until the head tiles are through before adding SWDGE
                # traffic - early ring contention delays the pipeline start.
                gp.wait_ge(gp_sem, 3)
                for t in sorted(SWDGE_TILES):
                    gp.dma_start(
                        out=xts[t][:], in_=_dram_tile(xf, t)
                    ).then_inc(dma_in[t], 16)

            @block.vector
            def _(vector):
                def fix(t):
                    # Column-31 poison repair (walrus rejects tensor ops on
                    # GpSimd, so this stays on the vector engine).
                    vector.tensor_tensor(
                        out=_col(qts[t], N_NODES - 1),
                        in0=_col(qts[t], N_NODES - 2),
                        in1=_col(xts[t], N_NODES - 1),
                        op=mybir.AluOpType.min,
                    ).then_inc(gp_sem, 1)

                vector.memset(cmask[:], NEG_BIG)
                vector.memset(_col(cmask, N_NODES - 1), POS_BIG)
                # Hazard: the fix reads the scan's freshly written tail;
                # run back-to-back after a SHORT (F=512) scan the strided
                # read samples stale SBUF.  Empirically immediate fixes are
                # clean for F >= 1024; defer only the short head tiles' fixes
                # by one scan.  gp_sem increments stay in tile order.
                pending = None
                for t in range(NT):
                    vector.wait_ge(dma_in[t], 16)
                    vector.tensor_tensor_scan(
                        out=qts[t][:],
                        data0=xts[t][:],
                        data1=cmask[:, : FSIZES[t]],
                        initial=POS_BIG,
                        op0=mybir.AluOpType.min,
                        op1=mybir.AluOpType.max,
                    )
                    if pending is not None:
                        fix(pending)
                        pending = None
                    if FSIZES[t] >= 1024:
                        fix(t)
                    else:
                        pending = t
                if pending is not None:
                    vector.tensor_copy(out=sep[:], in_=cmask[:, :64])
                    fix(pending)

            @block.scalar
            def _(scalar):
                # Dummy activation: pulls the sigmoid table load (~2.7us)
                # off the first tile's critical path.  Contents are unused,
                # so the uninitialized tile is fine.
                scalar.activation(
                    out=warm[:], in_=warm[:],
                    func=mybir.ActivationFunctionType.Sigmoid,
                )
                for t in range(NT):
                    scalar.wait_ge(gp_sem, t + 1)
                    scalar.activation(
                        out=qts[t][:],
                        in_=qts[t][:],
                        func=mybir.ActivationFunctionType.Sigmoid,
                    ).then_inc(act_sem, 1)
                    # The sequencer dispatches the DMA before the ACTIVATE's
                    # writes land; gate on its completion explicitly.
                    scalar.wait_ge(act_sem, t + 1)
                    scalar.dma_start(
                        out=_dram_tile(yf, t), in_=qts[t][:]
                    ).then_inc(dma_out, 16)
                scalar.wait_ge(dma_out, 16 * NT)

    return nc


def _run(x: np.ndarray, trace: bool = False):
    x = np.ascontiguousarray(np.asarray(x), dtype=np.float32)
    assert x.shape == (B_TOTAL, N_NODES), x.shape
    nc = _build()
    in_maps = [
        {"x": x[i * ROWS_PER_CORE : (i + 1) * ROWS_PER_CORE]} for i in range(N_CORES)
    ]
    res = run_bass_kernel_spmd(nc, in_maps, list(range(N_CORES)), trace=trace)
    out = np.concatenate([res.results[i]["y"] for i in range(N_CORES)], axis=0)
    return out, res


def _trn_devices_visible() -> bool:
    """True when this process' jax backend exposes the 8 NeuronCores.
    A caller that pinned jax to CPU (e.g. to run the reference) hides them;
    in that case the bass run must happen in a clean subprocess."""
    try:
        import jax

        return sum(1 for d in jax.devices() if d.platform != "cpu") >= N_CORES
    except Exception:
        return False


def _run_in_subprocess(x: np.ndarray) -> np.ndarray:
    with tempfile.TemporaryDirectory() as td:
        xin = os.path.join(td, "x.npy")
        xout = os.path.join(td, "y.npy")
        np.save(xin, x)
        env = dict(os.environ)
        for k in ("JAX_PLATFORMS", "JAX_PLATFORM_NAME"):
            env.pop(k, None)
        subprocess.run(
            [sys.executable, os.path.abspath(__file__), xin, xout],
            check=True,
            env=env,
        )
        return np.load(xout)


def kernel(x, children=None, child_mask=None, parents=None, parent_mask=None,
           topo=None, **_unused):
    x = np.ascontiguousarray(np.asarray(x), dtype=np.float32)
    if _trn_devices_visible():
        out, _ = _run(x)
        return out
    return _run_in_subprocess(x)


if __name__ == "__main__":
    _x = np.load(sys.argv[1])
    _out, _ = _run(_x)
    np.save(sys.argv[2], _out)



# revision 5
# speedup vs baseline: 1.1852x; 1.1852x over previous
"""DAG-constraint layer kernel for Trainium2 (8 NeuronCores, data parallel).

The reference computes p = sigmoid(x) followed by an iterative min/max
projection over a fixed chain+skip DAG on N=32 nodes (children of i are
{i+1, i+2}).  On that DAG the projection's fixed point is reached after a
single iteration and collapses to the prefix-min along the node axis:

    out[b, j] = min_{k <= j} sigmoid(x[b, k]) = sigmoid(cummin(x, axis=1))

(verified bitwise against the reference).  Since sigmoid is monotone it
commutes with the prefix-min, so the kernel computes
cummin(sigmoid(x)) - the sigmoid runs FIRST (scalar engine), the
prefix-min second (vector engine), which keeps the scalar engine ahead
of the vector engine in the pipeline.

Tolerance is 2e-2 relative; fp16 end-to-end gives ~5e-4 (sigmoid output
rel err 2^-11, and the fp16 rounding of x perturbs the result by at most
~4e-3 relative for the most negative x).  fp16 halves HBM traffic vs
fp32 - the kernel is memory-bound at ~425 GB/s/core combined in+out.

Layout: the host converts each core's shard (65536 rows x 32 nodes) to
fp16 and retiles it COLUMN-MAJOR per tile: a tile covering 128*G rows is
stored as [128 partitions][32 columns][G rows], so each column is a
G-element contiguous block in the free dimension.  In this layout the
32-wide prefix-min is a Brent-Kung scan over column BLOCKS: 9
tensor_tensor(min) instructions per tile (5 up-sweep + 4 down-sweep),
every operand a 3D AP with a G-contiguous inner dimension - which keeps
the DVE in its 2x fp16 perf mode (~0.53 ns/elem vs 2.13 ns/elem for the
hardware scan instruction, which has no fast mode).  No row-boundary
masks or fix-ups are needed: column blocks never straddle rows.

Pipeline per tile: SP ring DMAs the tile in (two half-tile DMAs so the
sigmoid can start on the first half early), ACT runs sigmoid (fp16 in /
fp16 out) and later issues the output DMA on its own ring, DVE runs the
9-op Brent-Kung chain in place.  The host undoes the tiling and upcasts
to fp32.

kernel() runs in-process when the 8 NeuronCores are visible to jax;
otherwise (e.g. the caller pinned jax to CPU) it re-executes itself in a
clean subprocess.
"""

import os
import subprocess
import sys
import tempfile
from contextlib import ExitStack

import numpy as np

import concourse.bass as bass
import concourse.mybir as mybir
from concourse.bass_utils import run_bass_kernel_spmd

N_CORES = 8
B_TOTAL = 524288
N_NODES = 32
ROWS_PER_CORE = B_TOTAL // N_CORES  # 65536
P = 128                             # SBUF partitions
# Rows-per-partition per tile (tile t covers P*G rows; sum must be 512).
# Small first tile shortens pipeline fill; small last tile shortens the
# BK + output-DMA drain.
GSIZES = [64, 128, 160, 160]
NT = len(GSIZES)
TOT_ELEMS = ROWS_PER_CORE * N_NODES  # 2097152 = P * 16384

assert sum(GSIZES) * P == ROWS_PER_CORE


def _build() -> bass.Bass:
    nc = bass.Bass()
    f16 = mybir.dt.float16
    MIN = mybir.AluOpType.min
    x = nc.declare_dram_parameter("x", [TOT_ELEMS], f16, isOutput=False)
    y = nc.declare_dram_parameter("y", [TOT_ELEMS], f16, isOutput=True)
    xf = x[:]
    yf = y[:]
    offs = [0]
    for g in GSIZES:
        offs.append(offs[-1] + P * N_NODES * g)

    def _dram_tile(flat, t):
        return flat[offs[t]: offs[t + 1]].rearrange("(p f) -> p f", p=P)

    with ExitStack() as es:
        ec = es.enter_context
        xts = [ec(nc.sbuf_tensor(f"xt{i}", [P, N_NODES * GSIZES[i]], f16))
               for i in range(NT)]
        sts = [ec(nc.sbuf_tensor(f"st{i}", [P, N_NODES * GSIZES[i]], f16))
               for i in range(NT)]
        warm = ec(nc.sbuf_tensor("act_warm", [P, 1], f16))
        # One semaphore per DMA: increments of different in-flight DMAs
        # interleave, so a shared counter is not a completion indicator.
        in_semA = [ec(nc.semaphore(f"in_semA{i}")) for i in range(NT)]
        in_semB = [ec(nc.semaphore(f"in_semB{i}")) for i in range(NT)]
        act_sem = ec(nc.semaphore("act_sem"))
        dve_sem = ec(nc.semaphore("dve_sem"))
        out_sem = ec(nc.semaphore("out_sem"))

        with nc.Block() as block:

            @block.sync
            def _(sync):
                # Two half-tile DMAs per tile: the sigmoid starts on the
                # first half while the second streams in.
                for t in range(NT):
                    full = _dram_tile(xf, t)
                    h = N_NODES * GSIZES[t] // 2
                    sync.dma_start(
                        out=xts[t][:, :h], in_=full[:, :h]
                    ).then_inc(in_semA[t], 16)
                    sync.dma_start(
                        out=xts[t][:, h:], in_=full[:, h:]
                    ).then_inc(in_semB[t], 16)

            @block.scalar
            def _(scalar):
                # Dummy activation pulls the sigmoid table load (~1.3us)
                # off the first tile's critical path.
                scalar.activation(
                    out=warm[:], in_=warm[:],
                    func=mybir.ActivationFunctionType.Sigmoid,
                )
                for t in range(NT):
                    h = N_NODES * GSIZES[t] // 2
                    scalar.wait_ge(in_semA[t], 16)
                    scalar.activation(
                        out=sts[t][:, :h], in_=xts[t][:, :h],
                        func=mybir.ActivationFunctionType.Sigmoid,
                    ).then_inc(act_sem, 1)
                    scalar.wait_ge(in_semB[t], 16)
                    scalar.activation(
                        out=sts[t][:, h:], in_=xts[t][:, h:],
                        func=mybir.ActivationFunctionType.Sigmoid,
                    ).then_inc(act_sem, 1)
                    # Output DMA for the previous tile (its BK chain is
                    # long done by now; the wait is usually satisfied).
                    if t > 0:
                        scalar.wait_ge(dve_sem, t)
                        scalar.dma_start(
                            out=_dram_tile(yf, t - 1), in_=sts[t - 1][:]
                        ).then_inc(out_sem, 16)
                scalar.wait_ge(dve_sem, NT)
                scalar.dma_start(
                    out=_dram_tile(yf, NT - 1), in_=sts[NT - 1][:]
                ).then_inc(out_sem, 16)
                scalar.wait_ge(out_sem, 16 * NT)

            @block.vector
            def _(vector):
                for t in range(NT):
                    g = GSIZES[t]
                    vector.wait_ge(act_sem, 2 * (t + 1))
                    # Brent-Kung min-scan over the 32 column blocks.
                    for k in range(1, 6):            # up-sweep
                        m = 1 << k
                        d = m >> 1
                        w = sts[t][:].rearrange(
                            "p (j2 m g) -> p j2 m g", m=m, g=g)
                        vector.tensor_tensor(
                            out=w[:, :, m - 1, :], in0=w[:, :, d - 1, :],
                            in1=w[:, :, m - 1, :], op=MIN)
                    for k in range(4, 0, -1):        # down-sweep
                        m = 1 << k
                        d = m >> 1
                        w = sts[t][:].rearrange(
                            "p (j2 m g) -> p j2 m g", m=m, g=g)
                        inst = vector.tensor_tensor(
                            out=w[:, 1:, d - 1, :], in0=w[:, :-1, m - 1, :],
                            in1=w[:, 1:, d - 1, :], op=MIN)
                    inst.then_inc(dve_sem, 1)

    return nc


def _host_pack(x: np.ndarray) -> list[np.ndarray]:
    """Per-core column-major fp16 tile streams."""
    x16 = x.astype(np.float16)
    streams = []
    for c in range(N_CORES):
        shard = x16[c * ROWS_PER_CORE:(c + 1) * ROWS_PER_CORE]
        parts = []
        r0 = 0
        for g in GSIZES:
            blk = shard[r0: r0 + P * g].reshape(P, g, N_NODES)
            parts.append(np.ascontiguousarray(blk.transpose(0, 2, 1)).reshape(-1))
            r0 += P * g
        streams.append(np.concatenate(parts))
    return streams


def _host_unpack(ys: list[np.ndarray]) -> np.ndarray:
    out = np.empty((B_TOTAL, N_NODES), dtype=np.float32)
    for c, ystream in enumerate(ys):
        r0 = c * ROWS_PER_CORE
        o0 = 0
        for g in GSIZES:
            n = P * N_NODES * g
            blk = ystream[o0: o0 + n].reshape(P, N_NODES, g)
            out[r0: r0 + P * g] = blk.transpose(0, 2, 1).reshape(P * g, N_NODES)
            r0 += P * g
            o0 += n
    return out


def _run(x: np.ndarray, trace: bool = False):
    x = np.asarray(x)
    assert x.shape == (B_TOTAL, N_NODES), x.shape
    nc = _build()
    streams = _host_pack(x)
    in_maps = [{"x": streams[i]} for i in range(N_CORES)]
    res = run_bass_kernel_spmd(nc, in_maps, list(range(N_CORES)), trace=trace)
    out = _host_unpack([np.asarray(res.results[i]["y"]) for i in range(N_CORES)])
    return out, res


def _trn_devices_visible() -> bool:
    """True when this process' jax backend exposes the 8 NeuronCores.
    A caller that pinned jax to CPU (e.g. to run the reference) hides them;
    in that case the bass run must happen in a clean subprocess."""
    try:
        import jax

        return sum(1 for d in jax.devices() if d.platform != "cpu") >= N_CORES
    except Exception:
        return False


def _run_in_subprocess(x: np.ndarray) -> np.ndarray:
    with tempfile.TemporaryDirectory() as td:
        xin = os.path.join(td, "x.npy")
        xout = os.path.join(td, "y.npy")
        np.save(xin, x)
        env = dict(os.environ)
        for k in ("JAX_PLATFORMS", "JAX_PLATFORM_NAME"):
            env.pop(k, None)
        subprocess.run(
            [sys.executable, os.path.abspath(__file__), xin, xout],
            check=True,
            env=env,
        )
        return np.load(xout)


def kernel(x, children=None, child_mask=None, parents=None, parent_mask=None,
           topo=None, **_unused):
    x = np.ascontiguousarray(np.asarray(x), dtype=np.float32)
    if _trn_devices_visible():
        out, _ = _run(x)
        return out
    return _run_in_subprocess(x)


if __name__ == "__main__":
    _x = np.load(sys.argv[1])
    _out, _ = _run(_x)
    np.save(sys.argv[2], _out)


# revision 6
# speedup vs baseline: 1.3783x; 1.1629x over previous
"""DAG-constraint layer kernel for Trainium2 (8 NeuronCores, data parallel).

The reference computes p = sigmoid(x) followed by an iterative min/max
projection over a fixed chain+skip DAG on N=32 nodes (children of i are
{i+1, i+2}).  On that DAG the projection's fixed point is reached after a
single iteration and collapses to the prefix-min along the node axis:

    out[b, j] = min_{k <= j} sigmoid(x[b, k]) = sigmoid(cummin(x, axis=1))

(verified bitwise against the reference).  Since sigmoid is monotone it
commutes with the prefix-min, so the kernel computes
cummin(sigmoid(x)) - the sigmoid runs FIRST (scalar engine), the
prefix-min second (vector engine), which keeps the scalar engine ahead
of the vector engine in the pipeline.

Tolerance is 2e-2 relative; fp16 end-to-end gives ~5e-4 (sigmoid output
rel err 2^-11, and the fp16 rounding of x perturbs the result by at most
~4e-3 relative for the most negative x).  fp16 halves HBM traffic vs
fp32 - the kernel is memory-bound at ~425 GB/s/core combined in+out.

Layout: the host converts each core's shard (65536 rows x 32 nodes) to
fp16 and retiles it COLUMN-MAJOR per tile: a tile covering 128*G rows is
stored as [128 partitions][32 columns][G rows], so each column is a
G-element contiguous block in the free dimension.  In this layout the
32-wide prefix-min is a Brent-Kung scan over column BLOCKS: 9
tensor_tensor(min) instructions per tile (5 up-sweep + 4 down-sweep),
every operand a 3D AP with a G-contiguous inner dimension - which keeps
the DVE in its 2x fp16 perf mode (~0.53 ns/elem vs 2.13 ns/elem for the
hardware scan instruction, which has no fast mode).  No row-boundary
masks or fix-ups are needed: column blocks never straddle rows.

Pipeline per tile: SP ring DMAs the tile in (two half-tile DMAs so the
sigmoid can start on the first half early), ACT runs sigmoid (fp16 in /
fp16 out) and later issues the output DMA on its own ring, DVE runs the
9-op Brent-Kung chain in place.  The host undoes the tiling and upcasts
to fp32.

kernel() runs in-process when the 8 NeuronCores are visible to jax;
otherwise (e.g. the caller pinned jax to CPU) it re-executes itself in a
clean subprocess.
"""

import os
import subprocess
import sys
import tempfile
from contextlib import ExitStack

import numpy as np

import concourse.bass as bass
import concourse.mybir as mybir
from concourse.bass_utils import run_bass_kernel_spmd

N_CORES = 8
B_TOTAL = 524288
N_NODES = 32
ROWS_PER_CORE = B_TOTAL // N_CORES  # 65536
P = 128                             # SBUF partitions
# Rows-per-partition per tile (tile t covers P*G rows; sum must be 512).
# Small first tile shortens pipeline fill; small last tile shortens the
# BK + output-DMA drain.
GSIZES = [64, 128, 160, 160]
NT = len(GSIZES)
TOT_ELEMS = ROWS_PER_CORE * N_NODES  # 2097152 = P * 16384

assert sum(GSIZES) * P == ROWS_PER_CORE


def _build() -> bass.Bass:
    nc = bass.Bass()
    f16 = mybir.dt.float16
    MIN = mybir.AluOpType.min
    x = nc.declare_dram_parameter("x", [TOT_ELEMS], f16, isOutput=False)
    y = nc.declare_dram_parameter("y", [TOT_ELEMS], f16, isOutput=True)
    xf = x[:]
    yf = y[:]
    offs = [0]
    for g in GSIZES:
        offs.append(offs[-1] + P * N_NODES * g)

    def _dram_tile(flat, t):
        return flat[offs[t]: offs[t + 1]].rearrange("(p f) -> p f", p=P)

    with ExitStack() as es:
        ec = es.enter_context
        xts = [ec(nc.sbuf_tensor(f"xt{i}", [P, N_NODES * GSIZES[i]], f16))
               for i in range(NT)]
        sts = [ec(nc.sbuf_tensor(f"st{i}", [P, N_NODES * GSIZES[i]], f16))
               for i in range(NT)]
        warm = ec(nc.sbuf_tensor("act_warm", [P, 1], f16))
        # One semaphore per input DMA: increments of different in-flight
        # DMAs interleave, so a shared counter is not a completion
        # indicator.
        in_sems = [ec(nc.semaphore(f"in_sem{i}")) for i in range(NT)]
        act_sem = ec(nc.semaphore("act_sem"))
        dve_sem = ec(nc.semaphore("dve_sem"))
        out_sem = ec(nc.semaphore("out_sem"))

        with nc.Block() as block:

            @block.sync
            def _(sync):
                # One DMA per tile: dma_start issuance costs ~0.8us each,
                # so fewer/bigger transfers fill the queue sooner.
                for t in range(NT):
                    sync.dma_start(
                        out=xts[t][:], in_=_dram_tile(xf, t)
                    ).then_inc(in_sems[t], 16)

            @block.scalar
            def _(scalar):
                # Dummy activation pulls the sigmoid table load (~1.3us)
                # off the first tile's critical path.
                scalar.activation(
                    out=warm[:], in_=warm[:],
                    func=mybir.ActivationFunctionType.Sigmoid,
                )
                for t in range(NT):
                    scalar.wait_ge(in_sems[t], 16)
                    scalar.activation(
                        out=sts[t][:], in_=xts[t][:],
                        func=mybir.ActivationFunctionType.Sigmoid,
                    ).then_inc(act_sem, 1)
                    # Output DMA for the previous tile (its BK chain is
                    # long done by now; the wait is usually satisfied).
                    if t > 0:
                        scalar.wait_ge(dve_sem, t)
                        scalar.dma_start(
                            out=_dram_tile(yf, t - 1), in_=sts[t - 1][:]
                        ).then_inc(out_sem, 16)
                scalar.wait_ge(dve_sem, NT)
                scalar.dma_start(
                    out=_dram_tile(yf, NT - 1), in_=sts[NT - 1][:]
                ).then_inc(out_sem, 16)
                scalar.wait_ge(out_sem, 16 * NT)

            @block.vector
            def _(vector):
                for t in range(NT):
                    g = GSIZES[t]
                    vector.wait_ge(act_sem, t + 1)
                    # Brent-Kung min-scan over the 32 column blocks.
                    for k in range(1, 6):            # up-sweep
                        m = 1 << k
                        d = m >> 1
                        w = sts[t][:].rearrange(
                            "p (j2 m g) -> p j2 m g", m=m, g=g)
                        vector.tensor_tensor(
                            out=w[:, :, m - 1, :], in0=w[:, :, d - 1, :],
                            in1=w[:, :, m - 1, :], op=MIN)
                    for k in range(4, 0, -1):        # down-sweep
                        m = 1 << k
                        d = m >> 1
                        w = sts[t][:].rearrange(
                            "p (j2 m g) -> p j2 m g", m=m, g=g)
                        inst = vector.tensor_tensor(
                            out=w[:, 1:, d - 1, :], in0=w[:, :-1, m - 1, :],
                            in1=w[:, 1:, d - 1, :], op=MIN)
                    inst.then_inc(dve_sem, 1)

    return nc


def _host_pack(x: np.ndarray) -> list[np.ndarray]:
    """Per-core column-major fp16 tile streams."""
    x16 = x.astype(np.float16)
    streams = []
    for c in range(N_CORES):
        shard = x16[c * ROWS_PER_CORE:(c + 1) * ROWS_PER_CORE]
        parts = []
        r0 = 0
        for g in GSIZES:
            blk = shard[r0: r0 + P * g].reshape(P, g, N_NODES)
            parts.append(np.ascontiguousarray(blk.transpose(0, 2, 1)).reshape(-1))
            r0 += P * g
        streams.append(np.concatenate(parts))
    return streams


def _host_unpack(ys: list[np.ndarray]) -> np.ndarray:
    out = np.empty((B_TOTAL, N_NODES), dtype=np.float32)
    for c, ystream in enumerate(ys):
        r0 = c * ROWS_PER_CORE
        o0 = 0
        for g in GSIZES:
            n = P * N_NODES * g
            blk = ystream[o0: o0 + n].reshape(P, N_NODES, g)
            out[r0: r0 + P * g] = blk.transpose(0, 2, 1).reshape(P * g, N_NODES)
            r0 += P * g
            o0 += n
    return out


def _run(x: np.ndarray, trace: bool = False):
    x = np.asarray(x)
    assert x.shape == (B_TOTAL, N_NODES), x.shape
    nc = _build()
    streams = _host_pack(x)
    in_maps = [{"x": streams[i]} for i in range(N_CORES)]
    res = run_bass_kernel_spmd(nc, in_maps, list(range(N_CORES)), trace=trace)
    out = _host_unpack([np.asarray(res.results[i]["y"]) for i in range(N_CORES)])
    return out, res


def _trn_devices_visible() -> bool:
    """True when this process' jax backend exposes the 8 NeuronCores.
    A caller that pinned jax to CPU (e.g. to run the reference) hides them;
    in that case the bass run must happen in a clean subprocess."""
    try:
        import jax

        return sum(1 for d in jax.devices() if d.platform != "cpu") >= N_CORES
    except Exception:
        return False


def _run_in_subprocess(x: np.ndarray) -> np.ndarray:
    with tempfile.TemporaryDirectory() as td:
        xin = os.path.join(td, "x.npy")
        xout = os.path.join(td, "y.npy")
        np.save(xin, x)
        env = dict(os.environ)
        for k in ("JAX_PLATFORMS", "JAX_PLATFORM_NAME"):
            env.pop(k, None)
        subprocess.run(
            [sys.executable, os.path.abspath(__file__), xin, xout],
            check=True,
            env=env,
        )
        return np.load(xout)


def kernel(x, children=None, child_mask=None, parents=None, parent_mask=None,
           topo=None, **_unused):
    x = np.ascontiguousarray(np.asarray(x), dtype=np.float32)
    if _trn_devices_visible():
        out, _ = _run(x)
        return out
    return _run_in_subprocess(x)


if __name__ == "__main__":
    _x = np.load(sys.argv[1])
    _out, _ = _run(_x)
    np.save(sys.argv[2], _out)
